# revision 33
# baseline (speedup 1.0000x reference)
"""TRN2 Bass kernel for nn_AttentionLayerDecoder (B=2, N=2048, HD=2048,
NH=16, KVH=4): RMSNorm -> GQA attention (inverted causal mask, no scaling)
-> output projection.

Sharding: 8 cores = (batch b in {0,1}) x (kv-group g in {0..3}).  Each core
computes 4 q-heads + its kv-head and a partial output projection
(contribution of its 512 columns of `a` through Wo); the host sums the 4
partials per batch.  All device tensors are pre-transposed on the host so
every matmul contraction sits on the partition axis; RMSNorm's norm_w is
folded into the weight matrices and the row scale s[n] (computed host-side,
shipped replicated across partitions) is applied to q/k/v out of PSUM.

Design notes (v3):
- Scores/Q/K stay f32r (attention is peaked; score rounding flips
  near-ties).  V / attention-weights / Wo / out ride bf16 (linear error
  only).
- Inverted-causal mask applied multiplicatively (ex *= {0,1} mask) on the
  DVE for the 4 diagonal-partial j-tiles per (head, i-chunk); fully-masked
  tiles are skipped, fully-allowed tiles need no mask.
- Softmax denominator: count matmul with an all-ones [128,128] lhsT so the
  per-i sums land replicated across all partitions -> reciprocal and
  normalize are plain elementwise DVE ops (no broadcast round-trips).
- Only Exp and Copy run on Act (one activation table, no reload churn).
- `a` never leaves SBUF; V transposed into [j, d] layout on the PE
  (identity matmul; dma_start_transpose is not dependency-tracked by the
  tile framework and races on HW).
- out is written bf16 (halves store traffic); host upcasts and sums.
- Row i=N-1 is fully masked -> uniform attention; its column is patched
  on the host (mean_j v = Wv @ mean(x*s)) and zeroed on device.
- Phases software-pipelined per 512-chunk: A(3) A(2) B(3) A(1) B(2) C(3)
  A(0) B(1) C(2) B(0) C(1) C(0); within B, scores run LAG=2 tiles ahead
  of the AV/count accumulation so the PE never waits on Act's exp.
- DMA issue spread across SP / Act / gpsimd queues (sim charges transfer
  time to the issuing queue): tok split SP/gpsimd, wq on Act, everything
  else (wk/wv/wo/s/masks/out) on SP.
"""
import numpy as np
from contextlib import ExitStack

import concourse.bass as bass
import concourse.tile as tile
from concourse import bacc, mybir
from concourse.bass_utils import run_bass_kernel_spmd

F32 = mybir.dt.float32
F32R = mybir.dt.float32r
BF16 = mybir.dt.bfloat16
BF16_NP = mybir.dt.np(mybir.dt.bfloat16)
AF = mybir.ActivationFunctionType
EPS = float(np.finfo(np.float32).eps)

B, N, HD = 2, 2048, 2048
NH, KVH = 16, 4
DD = HD // NH            # 128 head dim
H = NH // KVH            # 4 q-heads per kv-group / core
D = HD                   # model (contraction) dim
DOUT = HD
CH = 512                 # n/i chunk width (one PSUM bank at fp32)
N_CORES = 8
DT = D // 128            # 16 contraction tiles
NCH = N // CH            # 4 chunks
JT = N // 128            # 16 j tiles
JPC = CH // 128          # 4 j tiles per chunk
OT = DOUT // 128         # 16 output tiles
NMASK = CH // 128        # 4 diagonal mask variants
LAG = 2                  # B-phase software-pipeline depth


def _attention_kernel(ctx, tc, ext):
    nc = tc.nc

    cpool = ctx.enter_context(tc.tile_pool(name="consts", bufs=1))
    wpool = ctx.enter_context(tc.tile_pool(name="weights", bufs=1))
    big = ctx.enter_context(tc.tile_pool(name="big", bufs=1))
    tokp = ctx.enter_context(tc.tile_pool(name="tok", bufs=DT + 8))
    vtp = ctx.enter_context(tc.tile_pool(name="vt", bufs=2))
    stp = ctx.enter_context(tc.tile_pool(name="st", bufs=2))
    expp = ctx.enter_context(tc.tile_pool(name="expp", bufs=LAG + 4))
    obp = ctx.enter_context(tc.tile_pool(name="obp", bufs=3))
    psb = ctx.enter_context(tc.tile_pool(name="psb", bufs=1, space="PSUM"))

    # ---- consts ----
    ones_b = cpool.tile([128, 128], BF16, tag="ones_b")
    nc.gpsimd.memset(ones_b[:], 1.0)
    identr = cpool.tile([128, 128], F32, tag="identr")
    nc.sync.dma_start(identr[:], ext["identr"][:])
    mask_wide = cpool.tile([128, CH + (NMASK - 1) * 128], BF16, tag="mask_wide")
    nc.sync.dma_start(mask_wide[:], ext["masks"][:])
    masks = [mask_wide[:, (NMASK - 1 - d) * 128:(NMASK - 1 - d) * 128 + CH]
             for d in range(NMASK)]

    # ---- weights: wq on the Act queue; wk/wv go on SP inside A(3) ----
    wq_t = []
    for dt in range(DT):
        w = wpool.tile([128, H * DD], F32R, name=f"wq{dt}", tag=f"wq{dt}")
        nc.scalar.dma_start(w[:], ext["wq"][dt * 128:(dt + 1) * 128, :])
        wq_t.append(w)
    # wk/wv land in one strided DMA each ([2048,128] -> [128, 16*128]);
    # per-tile DMAs would each pay the 500ns descriptor-gen floor.
    wk_all = wpool.tile([128, DT * DD], F32R, tag="wk_all")
    wv_all = wpool.tile([128, DT * DD], F32R, tag="wv_all")
    wk_t = [wk_all[:, dt * DD:(dt + 1) * DD] for dt in range(DT)]
    wv_t = [wv_all[:, dt * DD:(dt + 1) * DD] for dt in range(DT)]
    wo_t = [wpool.tile([128, DOUT], BF16, name=f"wo{ht}", tag=f"wo{ht}")
            for ht in range(H)]

    qT = [big.tile([128, N], F32R, name=f"qT{e}", tag=f"qT{e}")
          for e in range(H)]
    kT = big.tile([128, N], F32R, tag="kT")
    # V^T per chunk, [128 j, 4*128 d].  NOTE: dma_start_transpose is NOT
    # dependency-tracked by the tile framework (races on HW); V transposes
    # go through the PE (identity matmul) instead.
    v4 = [big.tile([128, CH], BF16, name=f"v4_{c}", tag=f"v4_{c}")
          for c in range(NCH)]
    a_st = [big.tile([128, N], BF16, name=f"a{h}", tag=f"a{h}")
            for h in range(H)]

    tok_tiles = {}

    def tok_load(c):
        csl = slice(c * CH, (c + 1) * CH)
        tiles = []
        for dt in range(DT):
            t = tokp.tile([128, CH], F32R, tag="tok", name=f"tok{c}_{dt}")
            eng = nc.sync if dt % 2 == 0 else nc.gpsimd
            eng.dma_start(t[:], ext["tok"][dt * 128:(dt + 1) * 128, csl])
            tiles.append(t)
        tok_tiles[c] = tiles

    def phase_a(c, load_kv=False):
        csl = slice(c * CH, (c + 1) * CH)
        tok_c = tok_tiles.pop(c)
        if load_kv:
            nc.sync.dma_start(
                wk_all[:].rearrange("p (a c) -> p a c", a=DT),
                ext["wk"].rearrange("(a p) c -> p a c", p=128))
            nc.sync.dma_start(
                wv_all[:].rearrange("p (a c) -> p a c", a=DT),
                ext["wv"].rearrange("(a p) c -> p a c", p=128))
        s_t = stp.tile([128, CH], F32, tag="st", bufs=2)
        nc.sync.dma_start(s_t[:], ext["s"][:, csl])

        for e in range(H):
            ps_q = psb.tile([128, CH], F32, tag="mm", bufs=2)
            for dt in range(DT):
                nc.tensor.matmul(ps_q[:], wq_t[dt][:, e * 128:(e + 1) * 128],
                                 tok_c[dt][:],
                                 start=(dt == 0), stop=(dt == DT - 1))
            nc.vector.tensor_mul(qT[e][:, csl], ps_q[:], s_t[:])
        ps_k = psb.tile([128, CH], F32, tag="mm", bufs=2)
        for dt in range(DT):
            nc.tensor.matmul(ps_k[:], wk_t[dt][:], tok_c[dt][:],
                             start=(dt == 0), stop=(dt == DT - 1))
        nc.vector.tensor_mul(kT[:, csl], ps_k[:], s_t[:])
        ps_v = psb.tile([128, CH], F32, tag="mm", bufs=2)
        for dt in range(DT):
            nc.tensor.matmul(ps_v[:], wv_t[dt][:], tok_c[dt][:],
                             start=(dt == 0), stop=(dt == DT - 1))
        vt = vtp.tile([128, CH], F32, tag="vt", bufs=2)
        nc.vector.tensor_mul(vt[:], ps_v[:], s_t[:])
        ps_t = psb.tile([128, CH], F32, tag="mm", bufs=2)
        for js in range(JPC):
            nc.tensor.transpose(ps_t[:, js * 128:(js + 1) * 128],
                                vt[:, js * 128:(js + 1) * 128], identr[:])
        nc.scalar.copy(v4[c][:], ps_t[:])

    def phase_b(ic):
        isl = slice(ic * CH, (ic + 1) * CH)
        # descending j so the chain starts on a full tile (start=True must
        # cover the whole free range); diagonal partials come last with
        # their exp/mask/AV/count narrowed to the live column range.
        jts = [jt for jt in range(JT - 1, -1, -1) if 128 * jt + 127 > CH * ic]
        nst = len(jts)

        def cap_of(jt):
            t_off = CH * ic - 128 * jt
            if -CH < t_off < 127:
                d = -t_off // 128
                return min(CH, 128 * d + 127), d
            return CH, None

        for h in range(H):
            ps_av = psb.tile([128, CH], F32, tag="av", bufs=2)
            ps_cs = psb.tile([128, CH], F32, tag="cs", bufs=2)
            pend = {}
            for idx in range(nst + LAG):
                if idx < nst:
                    jt = jts[idx]
                    cap, d = cap_of(jt)
                    # f32r matmul is full-rate only at free>=256; round the
                    # scores width up to a 128 multiple >=256.
                    scw = min(CH, max(256, ((cap + 127) // 128) * 128))
                    ps_sc = psb.tile([128, CH], F32, tag="sc", bufs=2)
                    nc.tensor.matmul(ps_sc[:, :scw],
                                     kT[:, jt * 128:(jt + 1) * 128],
                                     qT[h][:, isl.start:isl.start + scw],
                                     start=True, stop=True)
                    ex = expp.tile([128, CH], BF16, tag="ex")
                    nc.scalar.activation(ex[:, :cap], ps_sc[:, :cap], AF.Exp)
                    if d is not None:
                        nc.vector.tensor_mul(ex[:, :cap], ex[:, :cap],
                                             masks[d][:, :cap])
                    pend[idx] = (ex, cap)
                if idx >= LAG:
                    j2 = idx - LAG
                    ex2, cap2 = pend.pop(j2)
                    jt2 = jts[j2]
                    first, last = (j2 == 0), (j2 == nst - 1)
                    nc.tensor.matmul(
                        ps_av[:, :cap2],
                        v4[jt2 // JPC][:, (jt2 % JPC) * 128:
                                       (jt2 % JPC) * 128 + 128],
                        ex2[:, :cap2], start=first, stop=last)
                    nc.tensor.matmul(ps_cs[:, :cap2], ones_b[:], ex2[:, :cap2],
                                     start=first, stop=last)
            if ic == NCH - 1:
                # column i=N-1 has no allowed j (and no tile writes it):
                # force denom 1 / sum 0 so the normalize writes 0 (the host
                # patches the real value).
                nc.vector.memset(ps_cs[:, CH - 1:CH], 1.0)
                nc.vector.memset(ps_av[:, CH - 1:CH], 0.0)
            rec = stp.tile([128, CH], F32, tag="rec", bufs=2)
            with nc.allow_low_precision(reason="softmax denom recip"):
                nc.vector.reciprocal(rec[:], ps_cs[:])
            nc.vector.tensor_mul(a_st[h][:, isl], ps_av[:], rec[:])

    def phase_c(c, load_wo=False):
        csl = slice(c * CH, (c + 1) * CH)
        if load_wo:
            for ht in range(H):
                nc.sync.dma_start(wo_t[ht][:], ext["wo"][ht * 128:(ht + 1) * 128, :])
        for o in range(OT):
            ps_o = psb.tile([128, CH], F32, tag="mm", bufs=2)
            for ht in range(H):
                nc.tensor.matmul(ps_o[:], wo_t[ht][:, o * 128:(o + 1) * 128],
                                 a_st[ht][:, csl],
                                 start=(ht == 0), stop=(ht == H - 1))
            ob = obp.tile([128, CH], BF16, tag="ob")
            nc.scalar.copy(ob[:], ps_o[:])
            nc.sync.dma_start(ext["out"][o * 128:(o + 1) * 128, csl], ob[:])

    tok_load(3)
    phase_a(3, load_kv=True)
    tok_load(2)
    phase_a(2)
    phase_b(3)
    tok_load(1)
    phase_a(1)
    phase_b(2)
    tok_load(0)
    phase_c(3, load_wo=True)
    phase_a(0)
    phase_b(1)
    phase_c(2)
    phase_b(0)
    phase_c(1)
    phase_c(0)
    return qT, kT, v4, a_st


def build_bass(reps=1):
    nc = bacc.Bacc("TRN2", target_bir_lowering=False, debug=False,
                   num_devices=N_CORES)
    ND = H * DD
    ext = {}
    ext["tok"] = nc.dram_tensor("tok", [D, N], F32R, kind="ExternalInput").ap()
    ext["s"] = nc.dram_tensor("s", [128, N], F32, kind="ExternalInput").ap()
    ext["wq"] = nc.dram_tensor("wq", [D, ND], F32R, kind="ExternalInput").ap()
    ext["wk"] = nc.dram_tensor("wk", [D, DD], F32R, kind="ExternalInput").ap()
    ext["wv"] = nc.dram_tensor("wv", [D, DD], F32R, kind="ExternalInput").ap()
    ext["wo"] = nc.dram_tensor("wo", [ND, DOUT], BF16, kind="ExternalInput").ap()
    ext["masks"] = nc.dram_tensor("masks", [128, CH + (NMASK - 1) * 128], BF16,
                                  kind="ExternalInput").ap()
    ext["identr"] = nc.dram_tensor("identr", [128, 128], F32,
                                   kind="ExternalInput").ap()
    ext["out"] = nc.dram_tensor("out", [DOUT, N], BF16, kind="ExternalOutput").ap()
    with tile.TileContext(nc) as tc:
        for _ in range(reps):
            with ExitStack() as ctx:
                _attention_kernel(ctx, tc, ext)
    nc.compile()
    return nc


def _make_masks():
    """Multiplicative inverted-causal masks, wide layout.  Slice d (the
    number of 128-row steps the j-tile sits below the i-chunk start) is
    mask_wide[:, (NMASK-1-d)*128 : +CH] with value 1 iff j>i, i.e.
    p > i_local - 128*d."""
    W = CH + (NMASK - 1) * 128
    p = np.arange(128)[:, None]
    u = np.arange(W)[None, :]
    wide = (p > u - (NMASK - 1) * 128).astype(np.float32)
    return wide.astype(BF16_NP)


def _rms_scale(tokens_b):
    ms = np.mean(tokens_b.astype(np.float32) ** 2, axis=-1) + EPS
    return (1.0 / np.sqrt(ms)).astype(np.float32)    # [N]


def make_in_maps(tokens, norm_w, Wq, Wk, Wv, Wo):
    """Per-core input dict list (core = b*KVH + g)."""
    masks = _make_masks()
    s_rep = [np.ascontiguousarray(
        np.broadcast_to(_rms_scale(tokens[b])[None, :], (128, N)))
        for b in range(B)]
    in_maps = []
    for core in range(N_CORES):
        b, g = divmod(core, KVH)
        # reference GQA: q-head h attends with kv-head h % KVH, so kv-group
        # g serves the interleaved q-heads {g, g+KVH, g+2*KVH, g+3*KVH}
        hidx = np.concatenate(
            [np.arange((g + KVH * j) * DD, (g + KVH * j + 1) * DD)
             for j in range(H)])
        in_maps.append({
            "tok": np.ascontiguousarray(tokens[b].T),
            "s": s_rep[b],
            "wq": np.ascontiguousarray((Wq[hidx] * norm_w[None, :]).T),
            "wk": np.ascontiguousarray(
                (Wk[g * DD:(g + 1) * DD] * norm_w[None, :]).T),
            "wv": np.ascontiguousarray(
                (Wv[g * DD:(g + 1) * DD] * norm_w[None, :]).T),
            "wo": np.ascontiguousarray(Wo[:, hidx].T).astype(BF16_NP),
            "masks": masks,
            "identr": np.eye(128, dtype=np.float32),
        })
    return in_maps


def assemble_out(core_outs, tokens, norm_w, Wv, bv, Wo, bo):
    """Sum per-core bf16 partials, add bo, and patch the fully-masked
    last row (uniform attention = mean_j v)."""
    out = np.zeros((B, N, HD), np.float32)
    for b in range(B):
        acc = np.zeros((DOUT, N), np.float32)
        for g in range(KVH):
            acc += np.asarray(core_outs[b * KVH + g]).astype(np.float32)
        ob = acc.T + bo[None, :]
        # host patch for row i=N-1: attention is uniform over all j
        s = _rms_scale(tokens[b])
        xbar = (tokens[b] * s[:, None] * norm_w[None, :]).mean(axis=0)
        vbar = xbar @ Wv.T + bv          # [KVD]
        a_last = np.tile(vbar, H)        # head h uses kv-head h % KVH
        ob[N - 1, :] = a_last @ Wo.T + bo
        out[b] = ob
    return out


_NC_CACHE = {}


def _get_nc():
    if "nc" not in _NC_CACHE:
        _NC_CACHE["nc"] = build_bass()
    return _NC_CACHE["nc"]


def _kernel_numpy(tokens, norm_w, Wq, bq, Wk, bk, Wv, bv, Wo, bo):
    """Reference-exact numpy fallback (used only if biases are nonzero,
    which the benchmark inputs never are)."""
    tokens = np.asarray(tokens, np.float32)
    x = tokens * (1.0 / np.sqrt((tokens ** 2).mean(-1, keepdims=True) + EPS))
    x = x * np.asarray(norm_w)[None, None, :]
    q = (x @ np.asarray(Wq).T + bq).reshape(B, N, NH, DD).transpose(0, 2, 1, 3)
    k = (x @ np.asarray(Wk).T + bk).reshape(B, N, KVH, DD).transpose(0, 2, 1, 3)
    v = (x @ np.asarray(Wv).T + bv).reshape(B, N, KVH, DD).transpose(0, 2, 1, 3)
    k = np.tile(k, (1, NH // KVH, 1, 1))
    v = np.tile(v, (1, NH // KVH, 1, 1))
    i = np.arange(N)
    mask = i[None, :] <= i[:, None]
    out = np.zeros((B, N, HD), np.float32)
    for b in range(B):
        for h in range(NH):
            sc = q[b, h] @ k[b, h].T
            sc = np.where(mask, np.float32(-1e9), sc)
            m = sc.max(1, keepdims=True)
            e = np.exp(sc - m)
            a = (e / e.sum(1, keepdims=True)) @ v[b, h]
            out[b, :, h * DD:(h + 1) * DD] = a
    return (out.reshape(B * N, HD) @ np.asarray(Wo).T + bo).reshape(B, N, HD)


def kernel(tokens, norm_w, Wq, bq, Wk, bk, Wv, bv, Wo, bo):
    tokens = np.asarray(tokens, np.float32)
    norm_w = np.asarray(norm_w, np.float32)
    Wq, Wk, Wv, Wo = (np.asarray(a, np.float32) for a in (Wq, Wk, Wv, Wo))
    bq, bk, bv, bo = (np.asarray(a, np.float32) for a in (bq, bk, bv, bo))
    if any(np.abs(b).max() > 0 for b in (bq, bk, bv)):
        # the device kernel folds norm into the weights, which only admits
        # zero q/k/v biases (benchmark inputs are zero-filled).
        return _kernel_numpy(tokens, norm_w, Wq, bq, Wk, bk, Wv, bv, Wo, bo)

    nc = _get_nc()
    in_maps = make_in_maps(tokens, norm_w, Wq, Wk, Wv, Wo)
    res = run_bass_kernel_spmd(nc, in_maps, core_ids=list(range(N_CORES)))
    return assemble_out([r["out"] for r in res.results],
                        tokens, norm_w, Wv, bv, Wo, bo)


# revision 34
# speedup vs baseline: 1.0585x; 1.0585x over previous
"""TRN2 Bass kernel for nn_AttentionLayerDecoder (B=2, N=2048, HD=2048,
NH=16, KVH=4): RMSNorm -> GQA attention (inverted causal mask, no scaling)
-> output projection.

Sharding: 8 cores = (batch b in {0,1}) x (kv-group g in {0..3}).  Each core
computes 4 q-heads + its kv-head and a partial output projection
(contribution of its 512 columns of `a` through Wo); the host sums the 4
partials per batch.  All device tensors are pre-transposed on the host so
every matmul contraction sits on the partition axis; RMSNorm's norm_w is
folded into the weight matrices and the row scale s[n] (computed host-side,
shipped replicated across partitions) is applied to q/k/v out of PSUM.

Design notes (v3):
- Scores/Q/K stay f32r (attention is peaked; score rounding flips
  near-ties).  V / attention-weights / Wo / out ride bf16 (linear error
  only).
- Inverted-causal mask applied multiplicatively (ex *= {0,1} mask) on the
  DVE for the 4 diagonal-partial j-tiles per (head, i-chunk); fully-masked
  tiles are skipped, fully-allowed tiles need no mask.
- Softmax denominator: count matmul with an all-ones [128,128] lhsT so the
  per-i sums land replicated across all partitions -> reciprocal and
  normalize are plain elementwise DVE ops (no broadcast round-trips).
- Only Exp and Copy run on Act (one activation table, no reload churn).
- `a` never leaves SBUF; V transposed into [j, d] layout on the PE
  (identity matmul; dma_start_transpose is not dependency-tracked by the
  tile framework and races on HW).
- out is written bf16 (halves store traffic); host upcasts and sums.
- Row i=N-1 is fully masked -> uniform attention; its column is patched
  on the host (mean_j v = Wv @ mean(x*s)) and zeroed on device.
- Phases software-pipelined per 512-chunk: A(3) A(2) B(3) A(1) B(2) C(3)
  A(0) B(1) C(2) B(0) C(1) C(0); within B, scores run LAG=2 tiles ahead
  of the AV/count accumulation so the PE never waits on Act's exp.
- DMA issue spread across SP / Act / gpsimd queues (sim charges transfer
  time to the issuing queue): tok split SP/gpsimd, wq on Act, everything
  else (wk/wv/wo/s/masks/out) on SP.
"""
import numpy as np
from contextlib import ExitStack

import concourse.bass as bass
import concourse.tile as tile
from concourse import bacc, mybir
from concourse.bass_utils import run_bass_kernel_spmd

F32 = mybir.dt.float32
F32R = mybir.dt.float32r
BF16 = mybir.dt.bfloat16
BF16_NP = mybir.dt.np(mybir.dt.bfloat16)
AF = mybir.ActivationFunctionType
EPS = float(np.finfo(np.float32).eps)

B, N, HD = 2, 2048, 2048
NH, KVH = 16, 4
DD = HD // NH            # 128 head dim
H = NH // KVH            # 4 q-heads per kv-group / core
D = HD                   # model (contraction) dim
DOUT = HD
CH = 512                 # n/i chunk width (one PSUM bank at fp32)
N_CORES = 8
DT = D // 128            # 16 contraction tiles
NCH = N // CH            # 4 chunks
JT = N // 128            # 16 j tiles
JPC = CH // 128          # 4 j tiles per chunk
OT = DOUT // 128         # 16 output tiles
NMASK = CH // 128        # 4 diagonal mask variants
LAG = 2                  # B-phase software-pipeline depth


def _attention_kernel(ctx, tc, ext):
    nc = tc.nc

    cpool = ctx.enter_context(tc.tile_pool(name="consts", bufs=1))
    wpool = ctx.enter_context(tc.tile_pool(name="weights", bufs=1))
    big = ctx.enter_context(tc.tile_pool(name="big", bufs=1))
    tokp = ctx.enter_context(tc.tile_pool(name="tok", bufs=DT + 8))
    vtp = ctx.enter_context(tc.tile_pool(name="vt", bufs=2))
    stp = ctx.enter_context(tc.tile_pool(name="st", bufs=2))
    expp = ctx.enter_context(tc.tile_pool(name="expp", bufs=LAG + 4))
    obp = ctx.enter_context(tc.tile_pool(name="obp", bufs=3))
    psb = ctx.enter_context(tc.tile_pool(name="psb", bufs=1, space="PSUM"))

    # ---- consts ----
    ones_b = cpool.tile([128, 128], BF16, tag="ones_b")
    nc.gpsimd.memset(ones_b[:], 1.0)
    identr = cpool.tile([128, 128], F32, tag="identr")
    nc.gpsimd.dma_start(identr[:], ext["identr"][:])
    mask_wide = cpool.tile([128, CH + (NMASK - 1) * 128], BF16, tag="mask_wide")
    nc.gpsimd.dma_start(mask_wide[:], ext["masks"][:])
    masks = [mask_wide[:, (NMASK - 1 - d) * 128:(NMASK - 1 - d) * 128 + CH]
             for d in range(NMASK)]

    # ---- weights: wq on the Act queue, one strided DMA per head so the
    # first q-chain starts after a quarter of the bytes; wk/wv go on SP
    # inside A(3) ----
    wq_e = []
    for e in range(H):
        w = wpool.tile([128, DT * 128], F32R, name=f"wqe{e}", tag=f"wqe{e}")
        nc.scalar.dma_start(
            w[:].rearrange("p (a c) -> p a c", a=DT),
            ext["wq"][:, e * 128:(e + 1) * 128].rearrange(
                "(a p) c -> p a c", p=128))
        wq_e.append(w)
    # wk/wv land in one strided DMA each ([2048,128] -> [128, 16*128]);
    # per-tile DMAs would each pay the 500ns descriptor-gen floor.
    wk_all = wpool.tile([128, DT * DD], F32R, tag="wk_all")
    wv_all = wpool.tile([128, DT * DD], F32R, tag="wv_all")
    wk_t = [wk_all[:, dt * DD:(dt + 1) * DD] for dt in range(DT)]
    wv_t = [wv_all[:, dt * DD:(dt + 1) * DD] for dt in range(DT)]
    wo_t = [wpool.tile([128, DOUT], BF16, name=f"wo{ht}", tag=f"wo{ht}")
            for ht in range(H)]

    qT = [big.tile([128, N], F32R, name=f"qT{e}", tag=f"qT{e}")
          for e in range(H)]
    kT = big.tile([128, N], F32R, tag="kT")
    # V^T per chunk, [128 j, 4*128 d].  NOTE: dma_start_transpose is NOT
    # dependency-tracked by the tile framework (races on HW); V transposes
    # go through the PE (identity matmul) instead.
    v4 = [big.tile([128, CH], BF16, name=f"v4_{c}", tag=f"v4_{c}")
          for c in range(NCH)]
    a_st = [big.tile([128, N], BF16, name=f"a{h}", tag=f"a{h}")
            for h in range(H)]

    tok_tiles = {}

    def tok_load(c):
        csl = slice(c * CH, (c + 1) * CH)
        tiles = []
        for dt in range(DT):
            t = tokp.tile([128, CH], F32R, tag="tok", name=f"tok{c}_{dt}")
            eng = nc.sync if dt % 2 == 0 else nc.gpsimd
            eng.dma_start(t[:], ext["tok"][dt * 128:(dt + 1) * 128, csl])
            tiles.append(t)
        tok_tiles[c] = tiles

    def phase_a(c, load_kv=False):
        csl = slice(c * CH, (c + 1) * CH)
        tok_c = tok_tiles.pop(c)
        if load_kv:
            nc.sync.dma_start(
                wk_all[:].rearrange("p (a c) -> p a c", a=DT),
                ext["wk"].rearrange("(a p) c -> p a c", p=128))
            nc.sync.dma_start(
                wv_all[:].rearrange("p (a c) -> p a c", a=DT),
                ext["wv"].rearrange("(a p) c -> p a c", p=128))
        s_t = stp.tile([128, CH], F32, tag="st", bufs=2)
        nc.sync.dma_start(s_t[:], ext["s"][:, csl])

        for e in range(H):
            ps_q = psb.tile([128, CH], F32, tag="mm", bufs=2)
            for dt in range(DT):
                nc.tensor.matmul(ps_q[:],
                                 wq_e[e][:, dt * 128:(dt + 1) * 128],
                                 tok_c[dt][:],
                                 start=(dt == 0), stop=(dt == DT - 1))
            nc.vector.tensor_mul(qT[e][:, csl], ps_q[:], s_t[:])
        ps_k = psb.tile([128, CH], F32, tag="mm", bufs=2)
        for dt in range(DT):
            nc.tensor.matmul(ps_k[:], wk_t[dt][:], tok_c[dt][:],
                             start=(dt == 0), stop=(dt == DT - 1))
        nc.vector.tensor_mul(kT[:, csl], ps_k[:], s_t[:])
        ps_v = psb.tile([128, CH], F32, tag="mm", bufs=2)
        for dt in range(DT):
            nc.tensor.matmul(ps_v[:], wv_t[dt][:], tok_c[dt][:],
                             start=(dt == 0), stop=(dt == DT - 1))
        vt = vtp.tile([128, CH], F32, tag="vt", bufs=2)
        nc.vector.tensor_mul(vt[:], ps_v[:], s_t[:])
        ps_t = psb.tile([128, CH], F32, tag="mm", bufs=2)
        for js in range(JPC):
            nc.tensor.transpose(ps_t[:, js * 128:(js + 1) * 128],
                                vt[:, js * 128:(js + 1) * 128], identr[:])
        nc.scalar.copy(v4[c][:], ps_t[:])

    def phase_b(ic):
        isl = slice(ic * CH, (ic + 1) * CH)
        # descending j so the chain starts on a full tile (start=True must
        # cover the whole free range); diagonal partials come last with
        # their exp/mask/AV/count narrowed to the live column range.
        jts = [jt for jt in range(JT - 1, -1, -1) if 128 * jt + 127 > CH * ic]
        nst = len(jts)

        def cap_of(jt):
            t_off = CH * ic - 128 * jt
            if -CH < t_off < 127:
                d = -t_off // 128
                return min(CH, 128 * d + 127), d
            return CH, None

        for h in range(H):
            ps_av = psb.tile([128, CH], F32, tag="av", bufs=2)
            ps_cs = psb.tile([128, CH], F32, tag="cs", bufs=2)
            pend = {}
            for idx in range(nst + LAG):
                if idx < nst:
                    jt = jts[idx]
                    cap, d = cap_of(jt)
                    # f32r matmul is full-rate only at free>=256; round the
                    # scores width up to a 128 multiple >=256.
                    scw = min(CH, max(256, ((cap + 127) // 128) * 128))
                    ps_sc = psb.tile([128, CH], F32, tag="sc", bufs=2)
                    nc.tensor.matmul(ps_sc[:, :scw],
                                     kT[:, jt * 128:(jt + 1) * 128],
                                     qT[h][:, isl.start:isl.start + scw],
                                     start=True, stop=True)
                    ex = expp.tile([128, CH], BF16, tag="ex")
                    nc.scalar.activation(ex[:, :cap], ps_sc[:, :cap], AF.Exp)
                    if d is not None:
                        nc.vector.tensor_mul(ex[:, :cap], ex[:, :cap],
                                             masks[d][:, :cap])
                    pend[idx] = (ex, cap)
                if idx >= LAG:
                    j2 = idx - LAG
                    ex2, cap2 = pend.pop(j2)
                    jt2 = jts[j2]
                    first, last = (j2 == 0), (j2 == nst - 1)
                    nc.tensor.matmul(
                        ps_av[:, :cap2],
                        v4[jt2 // JPC][:, (jt2 % JPC) * 128:
                                       (jt2 % JPC) * 128 + 128],
                        ex2[:, :cap2], start=first, stop=last)
                    nc.tensor.matmul(ps_cs[:, :cap2], ones_b[:], ex2[:, :cap2],
                                     start=first, stop=last)
            if ic == NCH - 1:
                # column i=N-1 has no allowed j (and no tile writes it):
                # force denom 1 / sum 0 so the normalize writes 0 (the host
                # patches the real value).
                nc.vector.memset(ps_cs[:, CH - 1:CH], 1.0)
                nc.vector.memset(ps_av[:, CH - 1:CH], 0.0)
            rec = stp.tile([128, CH], F32, tag="rec", bufs=2)
            with nc.allow_low_precision(reason="softmax denom recip"):
                nc.vector.reciprocal(rec[:], ps_cs[:])
            nc.vector.tensor_mul(a_st[h][:, isl], ps_av[:], rec[:])

    def phase_c(c, load_wo=False):
        csl = slice(c * CH, (c + 1) * CH)
        if load_wo:
            for ht in range(H):
                nc.sync.dma_start(wo_t[ht][:], ext["wo"][ht * 128:(ht + 1) * 128, :])
        for o in range(OT):
            ps_o = psb.tile([128, CH], F32, tag="mm", bufs=2)
            for ht in range(H):
                nc.tensor.matmul(ps_o[:], wo_t[ht][:, o * 128:(o + 1) * 128],
                                 a_st[ht][:, csl],
                                 start=(ht == 0), stop=(ht == H - 1))
            ob = obp.tile([128, CH], BF16, tag="ob")
            nc.scalar.copy(ob[:], ps_o[:])
            nc.sync.dma_start(ext["out"][o * 128:(o + 1) * 128, csl], ob[:])

    tok_load(3)
    phase_a(3, load_kv=True)
    tok_load(2)
    phase_a(2)
    phase_b(3)
    tok_load(1)
    phase_a(1)
    phase_b(2)
    tok_load(0)
    phase_c(3, load_wo=True)
    phase_a(0)
    phase_b(1)
    phase_c(2)
    phase_b(0)
    phase_c(1)
    phase_c(0)
    return qT, kT, v4, a_st


def build_bass(reps=1):
    nc = bacc.Bacc("TRN2", target_bir_lowering=False, debug=False,
                   num_devices=N_CORES)
    ND = H * DD
    ext = {}
    ext["tok"] = nc.dram_tensor("tok", [D, N], F32R, kind="ExternalInput").ap()
    ext["s"] = nc.dram_tensor("s", [128, N], F32, kind="ExternalInput").ap()
    ext["wq"] = nc.dram_tensor("wq", [D, ND], F32R, kind="ExternalInput").ap()
    ext["wk"] = nc.dram_tensor("wk", [D, DD], F32R, kind="ExternalInput").ap()
    ext["wv"] = nc.dram_tensor("wv", [D, DD], F32R, kind="ExternalInput").ap()
    ext["wo"] = nc.dram_tensor("wo", [ND, DOUT], BF16, kind="ExternalInput").ap()
    ext["masks"] = nc.dram_tensor("masks", [128, CH + (NMASK - 1) * 128], BF16,
                                  kind="ExternalInput").ap()
    ext["identr"] = nc.dram_tensor("identr", [128, 128], F32,
                                   kind="ExternalInput").ap()
    ext["out"] = nc.dram_tensor("out", [DOUT, N], BF16, kind="ExternalOutput").ap()
    with tile.TileContext(nc) as tc:
        for _ in range(reps):
            with ExitStack() as ctx:
                _attention_kernel(ctx, tc, ext)
    nc.compile()
    return nc


def _make_masks():
    """Multiplicative inverted-causal masks, wide layout.  Slice d (the
    number of 128-row steps the j-tile sits below the i-chunk start) is
    mask_wide[:, (NMASK-1-d)*128 : +CH] with value 1 iff j>i, i.e.
    p > i_local - 128*d."""
    W = CH + (NMASK - 1) * 128
    p = np.arange(128)[:, None]
    u = np.arange(W)[None, :]
    wide = (p > u - (NMASK - 1) * 128).astype(np.float32)
    return wide.astype(BF16_NP)


def _rms_scale(tokens_b):
    ms = np.mean(tokens_b.astype(np.float32) ** 2, axis=-1) + EPS
    return (1.0 / np.sqrt(ms)).astype(np.float32)    # [N]


def make_in_maps(tokens, norm_w, Wq, Wk, Wv, Wo):
    """Per-core input dict list (core = b*KVH + g)."""
    masks = _make_masks()
    s_rep = [np.ascontiguousarray(
        np.broadcast_to(_rms_scale(tokens[b])[None, :], (128, N)))
        for b in range(B)]
    in_maps = []
    for core in range(N_CORES):
        b, g = divmod(core, KVH)
        # reference GQA: q-head h attends with kv-head h % KVH, so kv-group
        # g serves the interleaved q-heads {g, g+KVH, g+2*KVH, g+3*KVH}
        hidx = np.concatenate(
            [np.arange((g + KVH * j) * DD, (g + KVH * j + 1) * DD)
             for j in range(H)])
        in_maps.append({
            "tok": np.ascontiguousarray(tokens[b].T),
            "s": s_rep[b],
            "wq": np.ascontiguousarray((Wq[hidx] * norm_w[None, :]).T),
            "wk": np.ascontiguousarray(
                (Wk[g * DD:(g + 1) * DD] * norm_w[None, :]).T),
            "wv": np.ascontiguousarray(
                (Wv[g * DD:(g + 1) * DD] * norm_w[None, :]).T),
            "wo": np.ascontiguousarray(Wo[:, hidx].T).astype(BF16_NP),
            "masks": masks,
            "identr": np.eye(128, dtype=np.float32),
        })
    return in_maps


def assemble_out(core_outs, tokens, norm_w, Wv, bv, Wo, bo):
    """Sum per-core bf16 partials, add bo, and patch the fully-masked
    last row (uniform attention = mean_j v)."""
    out = np.zeros((B, N, HD), np.float32)
    for b in range(B):
        acc = np.zeros((DOUT, N), np.float32)
        for g in range(KVH):
            acc += np.asarray(core_outs[b * KVH + g]).astype(np.float32)
        ob = acc.T + bo[None, :]
        # host patch for row i=N-1: attention is uniform over all j
        s = _rms_scale(tokens[b])
        xbar = (tokens[b] * s[:, None] * norm_w[None, :]).mean(axis=0)
        vbar = xbar @ Wv.T + bv          # [KVD]
        a_last = np.tile(vbar, H)        # head h uses kv-head h % KVH
        ob[N - 1, :] = a_last @ Wo.T + bo
        out[b] = ob
    return out


_NC_CACHE = {}


def _get_nc():
    if "nc" not in _NC_CACHE:
        _NC_CACHE["nc"] = build_bass()
    return _NC_CACHE["nc"]


def _kernel_numpy(tokens, norm_w, Wq, bq, Wk, bk, Wv, bv, Wo, bo):
    """Reference-exact numpy fallback (used only if biases are nonzero,
    which the benchmark inputs never are)."""
    tokens = np.asarray(tokens, np.float32)
    x = tokens * (1.0 / np.sqrt((tokens ** 2).mean(-1, keepdims=True) + EPS))
    x = x * np.asarray(norm_w)[None, None, :]
    q = (x @ np.asarray(Wq).T + bq).reshape(B, N, NH, DD).transpose(0, 2, 1, 3)
    k = (x @ np.asarray(Wk).T + bk).reshape(B, N, KVH, DD).transpose(0, 2, 1, 3)
    v = (x @ np.asarray(Wv).T + bv).reshape(B, N, KVH, DD).transpose(0, 2, 1, 3)
    k = np.tile(k, (1, NH // KVH, 1, 1))
    v = np.tile(v, (1, NH // KVH, 1, 1))
    i = np.arange(N)
    mask = i[None, :] <= i[:, None]
    out = np.zeros((B, N, HD), np.float32)
    for b in range(B):
        for h in range(NH):
            sc = q[b, h] @ k[b, h].T
            sc = np.where(mask, np.float32(-1e9), sc)
            m = sc.max(1, keepdims=True)
            e = np.exp(sc - m)
            a = (e / e.sum(1, keepdims=True)) @ v[b, h]
            out[b, :, h * DD:(h + 1) * DD] = a
    return (out.reshape(B * N, HD) @ np.asarray(Wo).T + bo).reshape(B, N, HD)


def kernel(tokens, norm_w, Wq, bq, Wk, bk, Wv, bv, Wo, bo):
    tokens = np.asarray(tokens, np.float32)
    norm_w = np.asarray(norm_w, np.float32)
    Wq, Wk, Wv, Wo = (np.asarray(a, np.float32) for a in (Wq, Wk, Wv, Wo))
    bq, bk, bv, bo = (np.asarray(a, np.float32) for a in (bq, bk, bv, bo))
    if any(np.abs(b).max() > 0 for b in (bq, bk, bv)):
        # the device kernel folds norm into the weights, which only admits
        # zero q/k/v biases (benchmark inputs are zero-filled).
        return _kernel_numpy(tokens, norm_w, Wq, bq, Wk, bk, Wv, bv, Wo, bo)

    nc = _get_nc()
    in_maps = make_in_maps(tokens, norm_w, Wq, Wk, Wv, Wo)
    res = run_bass_kernel_spmd(nc, in_maps, core_ids=list(range(N_CORES)))
    return assemble_out([r["out"] for r in res.results],
                        tokens, norm_w, Wv, bv, Wo, bo)


# revision 36
# speedup vs baseline: 1.0919x; 1.0315x over previous
"""TRN2 Bass kernel for nn_AttentionLayerDecoder (B=2, N=2048, HD=2048,
NH=16, KVH=4): RMSNorm -> GQA attention (inverted causal mask, no scaling)
-> output projection.

Sharding: 8 cores = (batch b in {0,1}) x (kv-group g in {0..3}).  Each core
computes 4 q-heads + its kv-head and a partial output projection
(contribution of its 512 columns of `a` through Wo); the host sums the 4
partials per batch.  All device tensors are pre-transposed on the host so
every matmul contraction sits on the partition axis; RMSNorm's norm_w is
folded into the weight matrices and the row scale s[n] (computed host-side,
shipped replicated across partitions) is applied to q/k/v out of PSUM.

Design notes (v3):
- Scores/Q/K stay f32r (attention is peaked; score rounding flips
  near-ties).  V / attention-weights / Wo / out ride bf16 (linear error
  only).
- Inverted-causal mask applied multiplicatively (ex *= {0,1} mask) on the
  DVE for the 4 diagonal-partial j-tiles per (head, i-chunk); fully-masked
  tiles are skipped, fully-allowed tiles need no mask.
- Softmax denominator: count matmul with an all-ones [128,128] lhsT so the
  per-i sums land replicated across all partitions -> reciprocal and
  normalize are plain elementwise DVE ops (no broadcast round-trips).
- Only Exp and Copy run on Act (one activation table, no reload churn).
- `a` never leaves SBUF; V transposed into [j, d] layout on the PE
  (identity matmul; dma_start_transpose is not dependency-tracked by the
  tile framework and races on HW).
- out is written bf16 (halves store traffic); host upcasts and sums.
- Row i=N-1 is fully masked -> uniform attention; its column is patched
  on the host (mean_j v = Wv @ mean(x*s)) and zeroed on device.
- Phases software-pipelined per 512-chunk: A(3) A(2) B(3) A(1) B(2) C(3)
  A(0) B(1) C(2) B(0) C(1) C(0); within B, scores run LAG=2 tiles ahead
  of the AV/count accumulation so the PE never waits on Act's exp.
- DMA issue spread across SP / Act / gpsimd queues (sim charges transfer
  time to the issuing queue): tok split SP/gpsimd, wq on Act, everything
  else (wk/wv/wo/s/masks/out) on SP.
"""
import numpy as np
from contextlib import ExitStack

import concourse.bass as bass
import concourse.tile as tile
from concourse import bacc, mybir
from concourse.bass_utils import run_bass_kernel_spmd

F32 = mybir.dt.float32
F32R = mybir.dt.float32r
BF16 = mybir.dt.bfloat16
BF16_NP = mybir.dt.np(mybir.dt.bfloat16)
AF = mybir.ActivationFunctionType
EPS = float(np.finfo(np.float32).eps)

B, N, HD = 2, 2048, 2048
NH, KVH = 16, 4
DD = HD // NH            # 128 head dim
H = NH // KVH            # 4 q-heads per kv-group / core
D = HD                   # model (contraction) dim
DOUT = HD
CH = 512                 # n/i chunk width (one PSUM bank at fp32)
N_CORES = 8
DT = D // 128            # 16 contraction tiles
NCH = N // CH            # 4 chunks
JT = N // 128            # 16 j tiles
JPC = CH // 128          # 4 j tiles per chunk
OT = DOUT // 128         # 16 output tiles
NMASK = CH // 128        # 4 diagonal mask variants
LAG = 2                  # B-phase software-pipeline depth


def _attention_kernel(ctx, tc, ext):
    nc = tc.nc

    cpool = ctx.enter_context(tc.tile_pool(name="consts", bufs=1))
    wpool = ctx.enter_context(tc.tile_pool(name="weights", bufs=1))
    big = ctx.enter_context(tc.tile_pool(name="big", bufs=1))
    tokp = ctx.enter_context(tc.tile_pool(name="tok", bufs=DT + 8))
    vtp = ctx.enter_context(tc.tile_pool(name="vt", bufs=2))
    stp = ctx.enter_context(tc.tile_pool(name="st", bufs=2))
    expp = ctx.enter_context(tc.tile_pool(name="expp", bufs=LAG + 4))
    obp = ctx.enter_context(tc.tile_pool(name="obp", bufs=3))
    psb = ctx.enter_context(tc.tile_pool(name="psb", bufs=1, space="PSUM"))

    # ---- consts ----
    ones_b = cpool.tile([128, 128], BF16, tag="ones_b")
    nc.gpsimd.memset(ones_b[:], 1.0)
    # ---- weights: wq on the Act queue, one strided DMA per head so the
    # first q-chain starts after a quarter of the bytes; wk/wv go on SP
    # inside A(3) ----
    wq_e = []
    for e in range(H):
        w = wpool.tile([128, DT * 128], F32R, name=f"wqe{e}", tag=f"wqe{e}")
        # head 0 rides the gpsimd queue: the Act queue opens with ~4us of
        # activation-table loads, which would delay the very first q-chain.
        eng = nc.gpsimd if e == 0 else nc.scalar
        eng.dma_start(
            w[:].rearrange("p (a c) -> p a c", a=DT),
            ext["wq"][:, e * 128:(e + 1) * 128].rearrange(
                "(a p) c -> p a c", p=128))
        wq_e.append(w)
    identr = cpool.tile([128, 128], F32, tag="identr")
    nc.gpsimd.dma_start(identr[:], ext["identr"][:])
    mask_wide = cpool.tile([128, CH + (NMASK - 1) * 128], BF16, tag="mask_wide")
    nc.gpsimd.dma_start(mask_wide[:], ext["masks"][:])
    masks = [mask_wide[:, (NMASK - 1 - d) * 128:(NMASK - 1 - d) * 128 + CH]
             for d in range(NMASK)]

    # wk/wv land in one strided DMA each ([2048,128] -> [128, 16*128]);
    # per-tile DMAs would each pay the 500ns descriptor-gen floor.
    wk_all = wpool.tile([128, DT * DD], F32R, tag="wk_all")
    wv_all = wpool.tile([128, DT * DD], F32R, tag="wv_all")
    wk_t = [wk_all[:, dt * DD:(dt + 1) * DD] for dt in range(DT)]
    wv_t = [wv_all[:, dt * DD:(dt + 1) * DD] for dt in range(DT)]
    wo_t = [wpool.tile([128, DOUT], BF16, name=f"wo{ht}", tag=f"wo{ht}")
            for ht in range(H)]

    qT = [big.tile([128, N], F32R, name=f"qT{e}", tag=f"qT{e}")
          for e in range(H)]
    kT = big.tile([128, N], F32R, tag="kT")
    # V^T per chunk, [128 j, 4*128 d].  NOTE: dma_start_transpose is NOT
    # dependency-tracked by the tile framework (races on HW); V transposes
    # go through the PE (identity matmul) instead.
    v4 = [big.tile([128, CH], BF16, name=f"v4_{c}", tag=f"v4_{c}")
          for c in range(NCH)]
    a_st = [big.tile([128, N], BF16, name=f"a{h}", tag=f"a{h}")
            for h in range(H)]

    tok_tiles = {}

    def tok_load(c):
        csl = slice(c * CH, (c + 1) * CH)
        tiles = []
        for dt in range(DT):
            t = tokp.tile([128, CH], F32R, tag="tok", name=f"tok{c}_{dt}")
            eng = nc.sync if dt % 2 == 0 else nc.gpsimd
            eng.dma_start(t[:], ext["tok"][dt * 128:(dt + 1) * 128, csl])
            tiles.append(t)
        tok_tiles[c] = tiles

    def phase_a(c, load_kv=False):
        csl = slice(c * CH, (c + 1) * CH)
        tok_c = tok_tiles.pop(c)
        if load_kv:
            nc.sync.dma_start(
                wk_all[:].rearrange("p (a c) -> p a c", a=DT),
                ext["wk"].rearrange("(a p) c -> p a c", p=128))
            nc.sync.dma_start(
                wv_all[:].rearrange("p (a c) -> p a c", a=DT),
                ext["wv"].rearrange("(a p) c -> p a c", p=128))
        s_t = stp.tile([128, CH], F32, tag="st", bufs=2)
        nc.sync.dma_start(s_t[:], ext["s"][:, csl])

        for e in range(H):
            ps_q = psb.tile([128, CH], F32, tag="mm", bufs=2)
            for dt in range(DT):
                nc.tensor.matmul(ps_q[:],
                                 wq_e[e][:, dt * 128:(dt + 1) * 128],
                                 tok_c[dt][:],
                                 start=(dt == 0), stop=(dt == DT - 1))
            nc.vector.tensor_mul(qT[e][:, csl], ps_q[:], s_t[:])
        ps_k = psb.tile([128, CH], F32, tag="mm", bufs=2)
        for dt in range(DT):
            nc.tensor.matmul(ps_k[:], wk_t[dt][:], tok_c[dt][:],
                             start=(dt == 0), stop=(dt == DT - 1))
        nc.vector.tensor_mul(kT[:, csl], ps_k[:], s_t[:])
        ps_v = psb.tile([128, CH], F32, tag="mm", bufs=2)
        for dt in range(DT):
            nc.tensor.matmul(ps_v[:], wv_t[dt][:], tok_c[dt][:],
                             start=(dt == 0), stop=(dt == DT - 1))
        vt = vtp.tile([128, CH], F32, tag="vt", bufs=2)
        nc.vector.tensor_mul(vt[:], ps_v[:], s_t[:])
        ps_t = psb.tile([128, CH], F32, tag="mm", bufs=2)
        for js in range(JPC):
            nc.tensor.transpose(ps_t[:, js * 128:(js + 1) * 128],
                                vt[:, js * 128:(js + 1) * 128], identr[:])
        nc.scalar.copy(v4[c][:], ps_t[:])

    def phase_b(ic):
        isl = slice(ic * CH, (ic + 1) * CH)
        # descending j so the chain starts on a full tile (start=True must
        # cover the whole free range); diagonal partials come last with
        # their exp/mask/AV/count narrowed to the live column range.
        jts = [jt for jt in range(JT - 1, -1, -1) if 128 * jt + 127 > CH * ic]
        nst = len(jts)

        def cap_of(jt):
            t_off = CH * ic - 128 * jt
            if -CH < t_off < 127:
                d = -t_off // 128
                return min(CH, 128 * d + 127), d
            return CH, None

        for h in range(H):
            ps_av = psb.tile([128, CH], F32, tag="av", bufs=2)
            ps_cs = psb.tile([128, CH], F32, tag="cs", bufs=2)
            pend = {}
            for idx in range(nst + LAG):
                if idx < nst:
                    jt = jts[idx]
                    cap, d = cap_of(jt)
                    # f32r matmul is full-rate only at free>=256; round the
                    # scores width up to a 128 multiple >=256.
                    scw = min(CH, max(256, ((cap + 127) // 128) * 128))
                    ps_sc = psb.tile([128, CH], F32, tag="sc", bufs=2)
                    nc.tensor.matmul(ps_sc[:, :scw],
                                     kT[:, jt * 128:(jt + 1) * 128],
                                     qT[h][:, isl.start:isl.start + scw],
                                     start=True, stop=True)
                    ex = expp.tile([128, CH], BF16, tag="ex")
                    nc.scalar.activation(ex[:, :cap], ps_sc[:, :cap], AF.Exp)
                    if d is not None:
                        nc.vector.tensor_mul(ex[:, :cap], ex[:, :cap],
                                             masks[d][:, :cap])
                    pend[idx] = (ex, cap)
                if idx >= LAG:
                    j2 = idx - LAG
                    ex2, cap2 = pend.pop(j2)
                    jt2 = jts[j2]
                    first, last = (j2 == 0), (j2 == nst - 1)
                    nc.tensor.matmul(
                        ps_av[:, :cap2],
                        v4[jt2 // JPC][:, (jt2 % JPC) * 128:
                                       (jt2 % JPC) * 128 + 128],
                        ex2[:, :cap2], start=first, stop=last)
                    nc.tensor.matmul(ps_cs[:, :cap2], ones_b[:], ex2[:, :cap2],
                                     start=first, stop=last)
            if ic == NCH - 1:
                # column i=N-1 has no allowed j (and no tile writes it):
                # force denom 1 / sum 0 so the normalize writes 0 (the host
                # patches the real value).
                nc.vector.memset(ps_cs[:, CH - 1:CH], 1.0)
                nc.vector.memset(ps_av[:, CH - 1:CH], 0.0)
            rec = stp.tile([128, CH], F32, tag="rec", bufs=2)
            with nc.allow_low_precision(reason="softmax denom recip"):
                nc.vector.reciprocal(rec[:], ps_cs[:])
            nc.vector.tensor_mul(a_st[h][:, isl], ps_av[:], rec[:])

    def phase_c(c, load_wo=False):
        csl = slice(c * CH, (c + 1) * CH)
        if load_wo:
            for ht in range(H):
                nc.sync.dma_start(wo_t[ht][:], ext["wo"][ht * 128:(ht + 1) * 128, :])
        for o in range(OT):
            ps_o = psb.tile([128, CH], F32, tag="mm", bufs=2)
            for ht in range(H):
                nc.tensor.matmul(ps_o[:], wo_t[ht][:, o * 128:(o + 1) * 128],
                                 a_st[ht][:, csl],
                                 start=(ht == 0), stop=(ht == H - 1))
            ob = obp.tile([128, CH], BF16, tag="ob")
            nc.scalar.copy(ob[:], ps_o[:])
            nc.sync.dma_start(ext["out"][o * 128:(o + 1) * 128, csl], ob[:])

    tok_load(3)
    phase_a(3, load_kv=True)
    tok_load(2)
    phase_a(2)
    phase_b(3)
    tok_load(1)
    phase_a(1)
    phase_b(2)
    tok_load(0)
    phase_c(3, load_wo=True)
    phase_a(0)
    phase_b(1)
    phase_c(2)
    phase_b(0)
    phase_c(1)
    phase_c(0)
    return qT, kT, v4, a_st


def build_bass(reps=1):
    nc = bacc.Bacc("TRN2", target_bir_lowering=False, debug=False,
                   num_devices=N_CORES)
    ND = H * DD
    ext = {}
    ext["tok"] = nc.dram_tensor("tok", [D, N], F32R, kind="ExternalInput").ap()
    ext["s"] = nc.dram_tensor("s", [128, N], F32, kind="ExternalInput").ap()
    ext["wq"] = nc.dram_tensor("wq", [D, ND], F32R, kind="ExternalInput").ap()
    ext["wk"] = nc.dram_tensor("wk", [D, DD], F32R, kind="ExternalInput").ap()
    ext["wv"] = nc.dram_tensor("wv", [D, DD], F32R, kind="ExternalInput").ap()
    ext["wo"] = nc.dram_tensor("wo", [ND, DOUT], BF16, kind="ExternalInput").ap()
    ext["masks"] = nc.dram_tensor("masks", [128, CH + (NMASK - 1) * 128], BF16,
                                  kind="ExternalInput").ap()
    ext["identr"] = nc.dram_tensor("identr", [128, 128], F32,
                                   kind="ExternalInput").ap()
    ext["out"] = nc.dram_tensor("out", [DOUT, N], BF16, kind="ExternalOutput").ap()
    with tile.TileContext(nc) as tc:
        for _ in range(reps):
            with ExitStack() as ctx:
                _attention_kernel(ctx, tc, ext)
    nc.compile()
    return nc


def _make_masks():
    """Multiplicative inverted-causal masks, wide layout.  Slice d (the
    number of 128-row steps the j-tile sits below the i-chunk start) is
    mask_wide[:, (NMASK-1-d)*128 : +CH] with value 1 iff j>i, i.e.
    p > i_local - 128*d."""
    W = CH + (NMASK - 1) * 128
    p = np.arange(128)[:, None]
    u = np.arange(W)[None, :]
    wide = (p > u - (NMASK - 1) * 128).astype(np.float32)
    return wide.astype(BF16_NP)


def _rms_scale(tokens_b):
    ms = np.mean(tokens_b.astype(np.float32) ** 2, axis=-1) + EPS
    return (1.0 / np.sqrt(ms)).astype(np.float32)    # [N]


def make_in_maps(tokens, norm_w, Wq, Wk, Wv, Wo):
    """Per-core input dict list (core = b*KVH + g)."""
    masks = _make_masks()
    s_rep = [np.ascontiguousarray(
        np.broadcast_to(_rms_scale(tokens[b])[None, :], (128, N)))
        for b in range(B)]
    in_maps = []
    for core in range(N_CORES):
        b, g = divmod(core, KVH)
        # reference GQA: q-head h attends with kv-head h % KVH, so kv-group
        # g serves the interleaved q-heads {g, g+KVH, g+2*KVH, g+3*KVH}
        hidx = np.concatenate(
            [np.arange((g + KVH * j) * DD, (g + KVH * j + 1) * DD)
             for j in range(H)])
        in_maps.append({
            "tok": np.ascontiguousarray(tokens[b].T),
            "s": s_rep[b],
            "wq": np.ascontiguousarray((Wq[hidx] * norm_w[None, :]).T),
            "wk": np.ascontiguousarray(
                (Wk[g * DD:(g + 1) * DD] * norm_w[None, :]).T),
            "wv": np.ascontiguousarray(
                (Wv[g * DD:(g + 1) * DD] * norm_w[None, :]).T),
            "wo": np.ascontiguousarray(Wo[:, hidx].T).astype(BF16_NP),
            "masks": masks,
            "identr": np.eye(128, dtype=np.float32),
        })
    return in_maps


def assemble_out(core_outs, tokens, norm_w, Wv, bv, Wo, bo):
    """Sum per-core bf16 partials, add bo, and patch the fully-masked
    last row (uniform attention = mean_j v)."""
    out = np.zeros((B, N, HD), np.float32)
    for b in range(B):
        acc = np.zeros((DOUT, N), np.float32)
        for g in range(KVH):
            acc += np.asarray(core_outs[b * KVH + g]).astype(np.float32)
        ob = acc.T + bo[None, :]
        # host patch for row i=N-1: attention is uniform over all j
        s = _rms_scale(tokens[b])
        xbar = (tokens[b] * s[:, None] * norm_w[None, :]).mean(axis=0)
        vbar = xbar @ Wv.T + bv          # [KVD]
        a_last = np.tile(vbar, H)        # head h uses kv-head h % KVH
        ob[N - 1, :] = a_last @ Wo.T + bo
        out[b] = ob
    return out


_NC_CACHE = {}


def _get_nc():
    if "nc" not in _NC_CACHE:
        _NC_CACHE["nc"] = build_bass()
    return _NC_CACHE["nc"]


def _kernel_numpy(tokens, norm_w, Wq, bq, Wk, bk, Wv, bv, Wo, bo):
    """Reference-exact numpy fallback (used only if biases are nonzero,
    which the benchmark inputs never are)."""
    tokens = np.asarray(tokens, np.float32)
    x = tokens * (1.0 / np.sqrt((tokens ** 2).mean(-1, keepdims=True) + EPS))
    x = x * np.asarray(norm_w)[None, None, :]
    q = (x @ np.asarray(Wq).T + bq).reshape(B, N, NH, DD).transpose(0, 2, 1, 3)
    k = (x @ np.asarray(Wk).T + bk).reshape(B, N, KVH, DD).transpose(0, 2, 1, 3)
    v = (x @ np.asarray(Wv).T + bv).reshape(B, N, KVH, DD).transpose(0, 2, 1, 3)
    k = np.tile(k, (1, NH // KVH, 1, 1))
    v = np.tile(v, (1, NH // KVH, 1, 1))
    i = np.arange(N)
    mask = i[None, :] <= i[:, None]
    out = np.zeros((B, N, HD), np.float32)
    for b in range(B):
        for h in range(NH):
            sc = q[b, h] @ k[b, h].T
            sc = np.where(mask, np.float32(-1e9), sc)
            m = sc.max(1, keepdims=True)
            e = np.exp(sc - m)
            a = (e / e.sum(1, keepdims=True)) @ v[b, h]
            out[b, :, h * DD:(h + 1) * DD] = a
    return (out.reshape(B * N, HD) @ np.asarray(Wo).T + bo).reshape(B, N, HD)


def kernel(tokens, norm_w, Wq, bq, Wk, bk, Wv, bv, Wo, bo):
    tokens = np.asarray(tokens, np.float32)
    norm_w = np.asarray(norm_w, np.float32)
    Wq, Wk, Wv, Wo = (np.asarray(a, np.float32) for a in (Wq, Wk, Wv, Wo))
    bq, bk, bv, bo = (np.asarray(a, np.float32) for a in (bq, bk, bv, bo))
    if any(np.abs(b).max() > 0 for b in (bq, bk, bv)):
        # the device kernel folds norm into the weights, which only admits
        # zero q/k/v biases (benchmark inputs are zero-filled).
        return _kernel_numpy(tokens, norm_w, Wq, bq, Wk, bk, Wv, bv, Wo, bo)

    nc = _get_nc()
    in_maps = make_in_maps(tokens, norm_w, Wq, Wk, Wv, Wo)
    res = run_bass_kernel_spmd(nc, in_maps, core_ids=list(range(N_CORES)))
    return assemble_out([r["out"] for r in res.results],
                        tokens, norm_w, Wv, bv, Wo, bo)
